# revision 96
# baseline (speedup 1.0000x reference)
"""Multi-head self-attention (RoPE, causal) Trainium2 Bass kernel.

Full inputs in, full output out. Sharding: 8 cores = 2 batch x 4 head-groups
(4 heads each). Per core: qkv projection, RoPE on DVE, streaming causal
attention (S^T orientation: softmax reduction along partitions via a
ones-column in V-hat), output projection partial. Host sums the 4 per-batch
partials and adds the (bv @ Wproj + bproj) constant.

Schedule: single fused pipeline over 512-token quarters. Attention for query
chunk qc only needs quarters <= qc, so its matmuls interleave with the NEXT
quarter's qkv matmuls in the PE stream — the qkv work fills the PE gaps left
by exp (ACT) latency, keeping the HAM clock gate at 8/8, and the attention
exps (the serial ACT bottleneck, ~95us) overlap the qkv phase. The per-qc
projection then interleaves with the last attention block the same way.

All matmul operands bf16 (fp32 PSUM accumulation). K stationaries are
zero-padded to 128 partitions: a 64-partition stationary never registers as
"busy" to the HAM clock gate and pins the PE at half clock. PV matmuls run
two k-tiles behind S so exp latency is off the PE critical path.

Self-contained: hardcodes all shapes for B=2, T=2048, D=1024, H=16, hd=64.
"""
from contextlib import ExitStack

import numpy as np

from concourse import bacc, mybir, tile
from concourse.bass_utils import run_bass_kernel_spmd
from concourse.dve_ops import RECIP_APPROX_FAST_CONSTS, RECIPROCAL_APPROX_FAST

f32 = mybir.dt.float32
f32r = mybir.dt.float32r
bf16 = mybir.dt.bfloat16
EXP = mybir.ActivationFunctionType.Exp

B, T, D = 2, 2048, 1024
H, HD = 16, 64
HALF = HD // 2  # 32
HPC = 4  # heads per core
BASE = 10000.0
NTQ = 4  # token quarters of 512
NQC = 4  # query chunks of 512
NKT = 16  # key tiles of 128
VW = HPC * (HD + 1)  # 260: v-hat columns per token tile
DEFER = 2  # PV matmuls run this many k-tiles behind S


def _build(dbg=False):
    nc = bacc.Bacc("TRN2", target_bir_lowering=False, debug=False, num_devices=8)

    xT = nc.dram_tensor("xT", [D, T], bf16, kind="ExternalInput").ap()
    wqk = nc.dram_tensor("wqk", [D, 512], bf16, kind="ExternalInput").ap()
    wv = nc.dram_tensor("wv", [D, 256], bf16, kind="ExternalInput").ap()
    wp = nc.dram_tensor("wp", [256, D], bf16, kind="ExternalInput").ap()
    bqk = nc.dram_tensor("bqk", [128, 4], f32, kind="ExternalInput").ap()
    cos4 = nc.dram_tensor("cos4", [128, T], bf16, kind="ExternalInput").ap()
    sin4 = nc.dram_tensor("sin4", [128, T], bf16, kind="ExternalInput").ap()
    trimask = nc.dram_tensor("trimask", [128, 128], bf16, kind="ExternalInput").ap()
    ones_pat = nc.dram_tensor("ones_pat", [128, 64], bf16, kind="ExternalInput").ap()
    outT = nc.dram_tensor("outT", [D, T], bf16, kind="ExternalOutput").ap()
    if dbg:
        dbg_vhat = nc.dram_tensor("dbg_vhat", [128, NKT * VW], bf16,
                                  kind="ExternalOutput").ap()
        dbg_qrA = nc.dram_tensor("dbg_qrA", [128, T], bf16,
                                 kind="ExternalOutput").ap()
        dbg_krp0 = nc.dram_tensor("dbg_krp0", [128, T], bf16,
                                  kind="ExternalOutput").ap()
        dbg_ot = nc.dram_tensor("dbg_ot", [128, T], bf16,
                                kind="ExternalOutput").ap()

    with tile.TileContext(nc) as tc, ExitStack() as ctx:
        consts = ctx.enter_context(tc.tile_pool(name="consts", bufs=1))
        wpool = ctx.enter_context(tc.tile_pool(name="wpool", bufs=1))
        xt_pool = ctx.enter_context(tc.tile_pool(name="xt", bufs=8))
        qkstage = ctx.enter_context(tc.tile_pool(name="qkstage", bufs=14))
        tmp_pool = ctx.enter_context(tc.tile_pool(name="tmp", bufs=4))
        vh_pool = ctx.enter_context(tc.tile_pool(name="vh", bufs=1))
        at_pool = ctx.enter_context(tc.tile_pool(name="at", bufs=5))
        small = ctx.enter_context(tc.tile_pool(name="small", bufs=8))
        rb_pool = ctx.enter_context(tc.tile_pool(name="rb", bufs=4))
        ot_pool = ctx.enter_context(tc.tile_pool(name="ot", bufs=2))
        ob_pool = ctx.enter_context(tc.tile_pool(name="ob", bufs=8))

        ps_qk = ctx.enter_context(tc.tile_pool(name="ps_qk", bufs=2, space="PSUM"))
        ps_v = ctx.enter_context(tc.tile_pool(name="ps_v", bufs=2, space="PSUM"))
        ps_s = ctx.enter_context(tc.tile_pool(name="ps_s", bufs=2, space="PSUM"))
        ps_o = ctx.enter_context(tc.tile_pool(name="ps_o", bufs=2, space="PSUM"))

        # ---- setup: weights/consts on the scalar queue, x rows split ----
        wqk_t = wpool.tile([128, 8, 512], bf16, tag="wqk_t")
        wv_t = wpool.tile([128, 8, 256], bf16, tag="wv_t")
        wp_t = wpool.tile([128, 2, D], bf16, tag="wp_t")
        cos_t = consts.tile([128, T], bf16, tag="cos_t")
        sin_t = consts.tile([128, T], bf16, tag="sin_t")
        tri_t = consts.tile([128, 128], bf16, tag="tri_t")
        bqk_t = consts.tile([128, 4], f32, tag="bqk_t")

        nc.scalar.dma_start(bqk_t[:], bqk)


        # v-hat: [128, 16 tok-tiles x (4 heads x 65)]; col 64 of each head = 1.0
        vhat = vh_pool.tile([128, NKT * VW], bf16, tag="vhat")
        vh_ones = vhat[:, :].rearrange("p (t h c) -> p t h c", t=NKT,
                                       h=HPC)[:, :, :, HD:HD + 1]
        nc.scalar.dma_start(vh_ones,
                            ones_pat.rearrange("p (t h) -> p t h", t=NKT)[:, :, :, None])

        # qkv stage + rope tensors [128, T]
        qE = qkstage.tile([128, T], bf16, tag="qks")
        qO = qkstage.tile([128, T], bf16, tag="qks")
        kE = qkstage.tile([128, T], bf16, tag="qks")
        kO = qkstage.tile([128, T], bf16, tag="qks")
        chunks = [qE, qO, kE, kO]
        qF = qkstage.tile([128, T], bf16, tag="qks")
        qS = qkstage.tile([128, T], bf16, tag="qks")
        kF = qkstage.tile([128, T], bf16, tag="qks")
        kS = qkstage.tile([128, T], bf16, tag="qks")
        qrA = qkstage.tile([128, T], bf16, tag="qks")
        qrB = qkstage.tile([128, T], bf16, tag="qks")
        # per-head K stationaries, zero-padded to the full 128 partitions
        kr_pad = [qkstage.tile([128, T], bf16, tag="qks",
                               name=f"krp{hh}") for hh in range(4)]

        # x loads: quarter-0 columns first so the first qkv wave starts
        # early, then the full-row remainder (triggers cost per row)
        # load order follows first use: quarter-0 x + the qE/qO weight halves
        # gate the first wave; k-halves, rope tables, then v/proj weights
        xc = []
        for dn in range(8):
            xt = xt_pool.tile([128, T], bf16, tag="xt", name=f"xt{dn}")
            nc.sync.dma_start(wqk_t[:, dn, 0:256],
                              wqk[dn * 128:(dn + 1) * 128, 0:256])
            eng = nc.sync if dn % 2 == 0 else nc.scalar
            eng.dma_start(xt[:, 0:512], xT[dn * 128:(dn + 1) * 128, 0:512])
            xc.append(xt)
        nc.sync.dma_start(cos_t[:], cos4)
        nc.scalar.dma_start(sin_t[:], sin4)
        for dn in range(8):
            nc.scalar.dma_start(wqk_t[:, dn, 256:512],
                                wqk[dn * 128:(dn + 1) * 128, 256:512])
        nc.scalar.dma_start(tri_t[:], trimask)
        for dn in range(8):
            nc.scalar.dma_start(wv_t[:, dn, :], wv[dn * 128:(dn + 1) * 128, :])
        for hd in range(2):
            nc.scalar.dma_start(wp_t[:, hd, :], wp[hd * 128:(hd + 1) * 128, :])
        for hh in range(4):
            r0 = (hh % 2) * 64
            nc.gpsimd.memset(kr_pad[hh][64 - r0:128 - r0, :], 0.0)

        otA = ot_pool.tile([128, T], bf16, tag="ot")
        otB = ot_pool.tile([128, T], bf16, tag="ot")
        # output staging: [128, 1024] half-tiles, recycled between T-halves
        obs = {}
        for half in range(2):
            for oc in range(8):
                obs[(half, oc)] = ob_pool.tile([128, 1024], bf16, tag="ob",
                                               name=f"ob{half}_{oc}")

        # ---- emitters ----
        def qkv_chunks(tq):
            """Quarter tq qkv matmuls in 2 psum waves; yields per dn step."""
            t0 = tq * 512
            sl = slice(t0, t0 + 512)
            if tq < 3:
                # prefetch next quarter's x on the sync queue
                n0 = (tq + 1) * 512
                for dn in range(8):
                    nc.sync.dma_start(xc[dn][:, n0:n0 + 512],
                                      xT[dn * 128:(dn + 1) * 128, n0:n0 + 512])

            def rope(E, O, F, S):
                t1 = tmp_pool.tile([128, 512], bf16, tag="tmp")
                t2 = tmp_pool.tile([128, 512], bf16, tag="tmp")
                nc.vector.tensor_mul(t1[:], E[:, sl], cos_t[:, sl])
                nc.vector.tensor_mul(t2[:], O[:, sl], sin_t[:, sl])
                nc.vector.tensor_sub(F[:, sl], t1[:], t2[:])
                t3 = tmp_pool.tile([128, 512], bf16, tag="tmp")
                t4 = tmp_pool.tile([128, 512], bf16, tag="tmp")
                nc.vector.tensor_mul(t3[:], E[:, sl], sin_t[:, sl])
                nc.vector.tensor_mul(t4[:], O[:, sl], cos_t[:, sl])
                nc.vector.tensor_add(S[:, sl], t3[:], t4[:])

            # wave 0: qE,qO matmuls -> bias -> rope-q -> q-permutes, so the
            # q side clears without waiting on k/v weight loads
            for wave in range(2):
                ch0 = 2 * wave
                pqk = [ps_qk.tile([128, 512], f32, tag="ps_qk",
                                  name=f"pqk{tq}_{ch0 + i}") for i in range(2)]
                for dn in range(8):
                    for i in range(2):
                        nc.tensor.matmul(
                            pqk[i][:],
                            wqk_t[:, dn, (ch0 + i) * 128:(ch0 + i + 1) * 128],
                            xc[dn][:, t0:t0 + 512],
                            start=(dn == 0), stop=(dn == 7))
                    yield
                for i in range(2):
                    ch = ch0 + i
                    nc.vector.tensor_scalar_add(
                        chunks[ch][:, t0:t0 + 512], pqk[i][:],
                        bqk_t[:, ch:ch + 1])
                if wave == 0:
                    rope(qE, qO, qF, qS)
                    for hh in range(4):
                        dst = qrA if hh < 2 else qrB
                        r0 = (hh % 2) * 64
                        nc.gpsimd.dma_start(dst[r0:r0 + 32, sl],
                                            qF[hh * 32:(hh + 1) * 32, sl])
                        nc.gpsimd.dma_start(dst[r0 + 32:r0 + 64, sl],
                                            qS[hh * 32:(hh + 1) * 32, sl])
                else:
                    rope(kE, kO, kF, kS)
                    for hh in range(4):
                        r0 = (hh % 2) * 64
                        nc.sync.dma_start(kr_pad[hh][r0:r0 + 32, sl],
                                          kF[hh * 32:(hh + 1) * 32, sl])
                        nc.sync.dma_start(kr_pad[hh][r0 + 32:r0 + 64, sl],
                                          kS[hh * 32:(hh + 1) * 32, sl])
                yield
            # wave 2: the v matmuls, one bank-aligned psum tile per tt group
            for sub in range(2):
                pvt = [ps_v.tile([128, 256], f32, tag="ps_v",
                                 name=f"pv{tq}_{2 * sub + i}")
                       for i in range(2)]
                for dn in range(8):
                    for i in range(2):
                        tt = 2 * sub + i
                        nc.tensor.matmul(
                            pvt[i][:],
                            xc[dn][:, t0 + tt * 128:t0 + (tt + 1) * 128],
                            wv_t[:, dn, :],
                            start=(dn == 0), stop=(dn == 7))
                    if dn % 2 == 1:
                        yield
                for i in range(2):
                    tg = tq * 4 + 2 * sub + i
                    dst = vhat[:, tg * VW:(tg + 1) * VW].rearrange(
                        "p (h c) -> p h c", h=HPC)[:, :, 0:HD]
                    nc.vector.tensor_copy(
                        dst, pvt[i][:].rearrange("p (h c) -> p h c", h=HPC))
                yield

        c = RECIP_APPROX_FAST_CONSTS

        def att_steps(qc):
            """Attention for query chunk qc, all heads; yields per S/PV step.
            PV runs DEFER k-tiles behind S to hide exp latency."""
            q0 = qc * 512
            nkt = 4 * qc + 4
            for h in range(HPC):
                qr = qrA if h < 2 else qrB
                kr = kr_pad[h]
                r0 = (h % 2) * 64
                po = ps_o.tile([65, 512], f32, tag="ps_o", name=f"po{h}_{qc}")
                den = small.tile([1, 512], f32, tag="den", name=f"den{h}_{qc}")

                pend = []

                def flush_pv():
                    kt, at, col_lo = pend.pop(0)
                    nc.tensor.matmul(
                        po[:, col_lo:512],
                        vhat[:, kt * VW + h * 65:kt * VW + (h + 1) * 65],
                        at[:, col_lo:512],
                        start=(kt == 0), stop=(kt == nkt - 1),
                        skip_group_check=True)
                    if kt == nkt - 1:
                        nc.vector.tensor_copy(den[:], po[64:65, :])

                for kt in range(nkt):
                    k0 = kt * 128
                    col_lo = k0 - q0 if k0 > q0 else 0  # diag sub-range
                    pss = ps_s.tile([128, 512], f32, tag="ps_s",
                                    name=f"pss{h}_{qc}_{kt}")
                    nc.tensor.matmul(
                        pss[:, col_lo:512],
                        kr[:, k0:k0 + 128],
                        qr[:, q0 + col_lo:q0 + 512],
                        start=True, stop=True)
                    at = at_pool.tile([128, 512], bf16, tag="at",
                                      name=f"at{h}_{qc}_{kt}")
                    nc.scalar.activation(at[:, col_lo:512],
                                         pss[:, col_lo:512], EXP)
                    if k0 >= q0:
                        nc.vector.tensor_mul(
                            at[:, col_lo:col_lo + 128],
                            at[:, col_lo:col_lo + 128], tri_t[:])
                    pend.append((kt, at, col_lo))
                    if len(pend) > DEFER:
                        flush_pv()
                    yield
                while pend:
                    flush_pv()
                    yield
                # normalize off the PE/ACT path: fast approx-recip, GpSimd
                # partition-broadcast, DVE multiply out of PSUM
                recip = small.tile([1, 512], f32, tag="recip",
                                   name=f"recip{h}_{qc}")
                nc.vector._custom_dve(
                    RECIPROCAL_APPROX_FAST, out=recip[:], in0=den[:],
                    s0=c["s0"], s1=c["s1"], imm2=c["imm2"])
                rb = rb_pool.tile([64, 512], f32, tag="rb",
                                  name=f"rb{h}_{qc}")
                nc.gpsimd.partition_broadcast(rb[:], recip[:])
                ot = otA if h < 2 else otB
                nc.vector.tensor_mul(ot[r0:r0 + 64, q0:q0 + 512],
                                     po[0:64, :], rb[:])
                yield

        def proj_chunks(qc):
            """Projection for query chunk qc; yields per oc."""
            q0 = qc * 512
            half = qc // 2
            c0 = (qc % 2) * 512  # column offset within the half tile
            for oc in range(8):
                pj = ps_qk.tile([128, 512], f32, tag="ps_qk",
                                name=f"pj{qc}_{oc}")
                nc.tensor.matmul(pj[:], wp_t[:, 0, oc * 128:(oc + 1) * 128],
                                 otA[:, q0:q0 + 512], start=True, stop=False)
                nc.tensor.matmul(pj[:], wp_t[:, 1, oc * 128:(oc + 1) * 128],
                                 otB[:, q0:q0 + 512], start=False, stop=True)
                ob = obs[(half, oc)]
                if oc % 2 == 0:
                    nc.vector.tensor_copy(ob[:, c0:c0 + 512], pj[:])
                else:
                    nc.scalar.copy(ob[:, c0:c0 + 512], pj[:])
                if qc == 1:
                    nc.sync.dma_start(
                        outT[oc * 128:(oc + 1) * 128, 0:1024], ob[:])
                elif qc >= 2:
                    # drain each 512-col block as soon as it exists so the
                    # final drain is only qc3's megabyte, split over queues
                    deng = nc.sync if (qc == 2 or oc % 2 == 0) else nc.scalar
                    deng.dma_start(
                        outT[oc * 128:(oc + 1) * 128, q0:q0 + 512],
                        ob[:, c0:c0 + 512])
                yield

        def drain(gen):
            for _ in gen:
                pass

        def interleave(ga, gb, ratio, delay=0):
            """Pull all of ga and gb, taking `ratio` steps of gb per ga step
            once `delay` ga-steps have been emitted."""
            acc = 0.0
            ga, gb = iter(ga), iter(gb)
            a_live = b_live = True
            n_a = 0
            while a_live or b_live:
                if a_live:
                    try:
                        next(ga)
                        n_a += 1
                    except StopIteration:
                        a_live = False
                if n_a >= delay:
                    acc += ratio
                while b_live and acc >= 1.0:
                    acc -= 1.0
                    try:
                        next(gb)
                    except StopIteration:
                        b_live = False
                if not a_live and b_live:
                    try:
                        next(gb)
                    except StopIteration:
                        b_live = False

        # ---- the fused pipeline ----
        import os as _os
        _seq = _os.environ.get("KSEQ") == "1"
        if _seq:
            for _tq in range(4):
                drain(qkv_chunks(_tq))
            for _qc in range(3):
                drain(att_steps(_qc))
        else:
            drain(qkv_chunks(0))
            interleave(qkv_chunks(1), att_steps(0), ratio=2.8, delay=18)
            interleave(qkv_chunks(2), att_steps(1), ratio=4.4, delay=18)
            interleave(qkv_chunks(3), att_steps(2), ratio=6.0, delay=18)

        def proj_all():
            for qc in range(3):
                for _ in proj_chunks(qc):
                    yield

        if _seq:
            drain(att_steps(3))
            for _qc in range(3):
                drain(proj_chunks(_qc))
        else:
            interleave(att_steps(3), proj_all(), ratio=0.33)
        drain(proj_chunks(3))

        if dbg:
            nc.scalar.dma_start(dbg_vhat, vhat[:])
            nc.scalar.dma_start(dbg_qrA, qrA[:])
            nc.scalar.dma_start(dbg_krp0, kr_pad[0][:])
            nc.scalar.dma_start(dbg_ot, otA[:])

    nc.compile()
    return nc


_NC = None


def _get_nc():
    global _NC
    if _NC is None:
        _NC = _build()
    return _NC


def _host_prep(x, Wqkv, bqkv, Wproj, bproj, pos):
    """Build the 8 per-core input maps."""
    import ml_dtypes
    nbf16 = ml_dtypes.bfloat16

    x = np.asarray(x, dtype=np.float32)
    Wqkv = np.asarray(Wqkv, dtype=np.float32)
    bqkv = np.asarray(bqkv, dtype=np.float32)
    Wproj = np.asarray(Wproj, dtype=np.float32)
    bproj = np.asarray(bproj, dtype=np.float32)
    pos = int(np.asarray(pos))

    scale = HD ** -0.5
    # rope tables, layout [128 = 4 heads x 32 thetas (h-major), T]
    theta = 1.0 / BASE ** (np.arange(HALF, dtype=np.float32) / HALF)
    angles = np.outer(np.arange(pos, pos + T, dtype=np.float32), theta)  # [T,32]
    cosT = np.cos(angles).T.astype(np.float32)  # [32, T]
    sinT = np.sin(angles).T.astype(np.float32)
    cos4 = np.ascontiguousarray(np.tile(cosT, (4, 1))).astype(nbf16)  # [128, T]
    sin4 = np.ascontiguousarray(np.tile(sinT, (4, 1))).astype(nbf16)

    tri = np.tril(np.ones((128, 128), dtype=np.float32)).T  # m[p,j]=1 if p<=j
    tri = np.ascontiguousarray(tri).astype(nbf16)

    in_maps = []
    for c in range(8):
        b, hg = c // 4, c % 4
        heads = [4 * hg + h for h in range(HPC)]
        permE = np.array([h * HD + 2 * i for h in heads for i in range(HALF)])
        permO = permE + 1
        wqk_np = np.concatenate([
            Wqkv[:, permE] * scale,          # qE
            Wqkv[:, permO] * scale,          # qO
            Wqkv[:, D + permE],              # kE
            Wqkv[:, D + permO],              # kO
        ], axis=1)
        bqk_np = np.stack([
            bqkv[permE] * scale, bqkv[permO] * scale,
            bqkv[D + permE], bqkv[D + permO],
        ], axis=1)
        wv_np = Wqkv[:, 2 * D + 256 * hg: 2 * D + 256 * (hg + 1)]
        wp_np = Wproj[256 * hg: 256 * (hg + 1), :]
        in_maps.append({
            "xT": np.ascontiguousarray(x[b].T).astype(nbf16),
            "wqk": np.ascontiguousarray(wqk_np).astype(nbf16),
            "wv": np.ascontiguousarray(wv_np).astype(nbf16),
            "wp": np.ascontiguousarray(wp_np).astype(nbf16),
            "bqk": np.ascontiguousarray(bqk_np, dtype=np.float32),
            "cos4": cos4,
            "sin4": sin4,
            "trimask": tri,
            "ones_pat": np.ones((128, 64), dtype=nbf16),
        })
    const_vec = bqkv[2 * D:] @ Wproj + bproj  # exact host-side bias handling
    return in_maps, const_vec


def kernel(x, Wqkv, bqkv, Wproj, bproj, pos, **kw):
    in_maps, const_vec = _host_prep(x, Wqkv, bqkv, Wproj, bproj, pos)
    nc = _get_nc()
    res = run_bass_kernel_spmd(nc, in_maps, core_ids=list(range(8))).results
    out = np.empty((B, T, D), dtype=np.float32)
    for b in range(B):
        acc = res[4 * b]["outT"].astype(np.float32)
        for c in range(4 * b + 1, 4 * b + 4):
            acc += res[c]["outT"].astype(np.float32)
        out[b] = acc.T + const_vec
    return out
